# revision 1
# baseline (speedup 1.0000x reference)
"""Causal bag-of-words kernel for Trainium2 (8 NeuronCores, SPMD).

out[b, t, :] = mean(x[b, :t+1, :], axis=0)  for x of shape (8, 8192, 512) f32.

Sharding: data-parallel over B — core b handles x[b] (8192, 512) independently.

Per-core algorithm (natural [t, c] layout, no transposes):
  T = 8192 is split into 64 blocks of 128 rows (partition dim).
  For block k with rows X_k [128, 512]:
    psum_k = U @ Xr_k + J @ Zr_{k-1}     (two full-rate fp32r PE matmuls)
  where U is upper-triangular ones (cumsum within the block), J is all-ones
  (broadcasts the column-sum of Z over all 128 rows), Xr_k is X_k rounded to
  fp32r (one DVE copy), and Zr_{k-1} rounds the exact f32 running sum of
  blocks 0..k-1.  fp32r (TF32-like) runs 1 cycle/row at N=512 vs 4 for ieee
  fp32 — trading ~2^-12 relative error for a 4x PE-time cut (tolerance is
  2e-2).  Precision structure:
   - The running sum is kept EXACT in f32 as a two-level prefix (within-wave
     prefixes s_j on a 7-add serial DVE chain per wave, plus a once-per-wave
     wave-total W), and each carry operand Zr is rounded ONCE from exact
     operands — avoiding the biased ~2^-12*sqrt(k) random walk a rounded
     63-step chain accumulates.
   - Block 0 (counts 1..128) and the carries of wave 0 (counts < 1024) are
     computed exactly via hi/lo fp32r splits, reusing the free carry-matmul
     slot of block 0 and one extra carry matmul for blocks 1..8.
  The 1/(t+1) scaling is folded into the PSUM->SBUF evacuation (ACT
  activation with a per-partition scale vector), which also narrows the
  output to bf16 — halving store-side HBM traffic (the kernel is
  memory-bound; ~2^-9 output rounding is far inside the tolerance).  The
  host upcasts back to f32.
  Blocks stream in waves of 8 (2 MiB input DMAs on the SP HWDGE ring);
  stores go per half-wave on the ACT HWDGE ring.  Elementwise work stays on
  DVE (+ACT for evacuation): GPSIMD tensor ops and ACT casts measured 1.5-2x
  slower on real hardware despite favorable cost-model predictions.
"""

import sys

sys.path.insert(0, "/opt/trn_rl_repo")

import numpy as np

import concourse.bacc as bacc
import concourse.bass as bass
import concourse.mybir as mybir
import concourse.tile as tile
from concourse.bass_utils import run_bass_kernel_spmd

B, T, C = 8, 8192, 512
P = 128                 # partition dim / block size along T
NB = T // P             # 64 blocks
G = 8                   # blocks per wave (2 MiB per input DMA)
NW = NB // G            # 8 waves
N_CORES = 8
F32 = mybir.dt.float32
F32R = mybir.dt.float32r  # full-rate fp32 matmul path (4x faster at N>=256)
BF16 = mybir.dt.bfloat16

_cache: dict = {}


def build_program(n_iter: int = 1, loop_n: int = 1, g: int = 8,
                  out_dt: str = "bf16", cast_engine: str = "vector",
                  zdt: str = "f32r", evac_engine: str = "scalar",
                  pair_cast: bool = False, store_mode: str = "half",
                  in_ring: str = "sync", zr_engine: str = "vector",
                  xin_bufs: int = 4, obufs: int = 3, zbufs: int = 8,
                  xrbufs: int = 8, psbufs: int = 8):
    """Build + compile the per-core Bass program (SPMD, identical on all cores).

    n_iter > 1 unrolls the whole computation; loop_n > 1 wraps it in a
    hardware For_i loop (both for timing by the slope method); results are
    identical for any value.
    cast_engine: engine that pre-rounds x to fp32r (the BIR verifier requires
    every fp32r-matmul operand's producer to write rounded F32R).
    zdt: dtype of the running block-sum chain ('f32r' feeds the carry matmul
    directly; 'f32' is invalid for the fp32r path).
    out_dt: 'bf16' (half store traffic; host upcasts) or 'f32'.
    """
    G = g
    NW = NB // G
    nc = bacc.Bacc("TRN2", target_bir_lowering=False, debug=False,
                   num_devices=N_CORES)
    OUT_DT = BF16 if out_dt == "bf16" else F32
    ZDT = F32R if zdt == "f32r" else F32

    x_d = nc.dram_tensor("x", [T, C], F32, kind="ExternalInput")
    u_d = nc.dram_tensor("u", [P, P], F32, kind="ExternalInput")
    j_d = nc.dram_tensor("jm", [P, P], F32, kind="ExternalInput")
    r_d = nc.dram_tensor("recip", [P, NB], F32, kind="ExternalInput")
    o_d = nc.dram_tensor("out", [T, C], OUT_DT, kind="ExternalOutput")

    ACT_COPY = mybir.ActivationFunctionType.Copy
    cast_cycle = cast_engine.split(",")
    evac_cycle = evac_engine.split(",")
    zr_cycle = zr_engine.split(",")
    cnt = {"cast": 0, "evac": 0, "zr": 0}

    def cast_r(dst, src):
        eng = cast_cycle[cnt["cast"] % len(cast_cycle)]
        cnt["cast"] += 1
        if eng == "scalar":
            nc.scalar.activation(dst, src, ACT_COPY)
        else:
            getattr(nc, eng).tensor_copy(dst, src)

    def zr_add(dst, a, b):
        eng = zr_cycle[cnt["zr"] % len(zr_cycle)]
        cnt["zr"] += 1
        getattr(nc, eng).tensor_add(dst, a, b)

    def evac(dst, ps_ap, r_ap):
        eng = evac_cycle[cnt["evac"] % len(evac_cycle)]
        cnt["evac"] += 1
        if eng == "scalar":
            nc.scalar.activation(dst, ps_ap, ACT_COPY, scale=r_ap)
        else:
            getattr(nc, eng).tensor_scalar_mul(dst, ps_ap, r_ap)

    with tile.TileContext(nc) as tc:
        with (
            tc.tile_pool(name="consts", bufs=1) as consts,
            tc.tile_pool(name="xin", bufs=xin_bufs) as xin,
            tc.tile_pool(name="oput", bufs=obufs) as oput,
            tc.tile_pool(name="zp", bufs=zbufs) as zp,
            tc.tile_pool(name="xr", bufs=xrbufs) as xrp,
            tc.tile_pool(name="sp", bufs=12) as sp,
            tc.tile_pool(name="wp", bufs=3) as wp,
            tc.tile_pool(name="ps", bufs=psbufs, space="PSUM") as psp,
        ):
            # consts go via SWDGE (gpsimd) so the HWDGE rings start on the
            # first wave load immediately
            u_t = consts.tile([P, P], F32)
            j_t = consts.tile([P, P], F32)
            r_t = consts.tile([P, NB], F32)
            nc.gpsimd.dma_start(u_t[:], u_d[:])
            nc.gpsimd.dma_start(j_t[:], j_d[:])
            nc.gpsimd.dma_start(r_t[:], r_d[:])
            # 0/1 matrices are exact in fp32r
            u_r = consts.tile([P, P], F32R)
            j_r = consts.tile([P, P], F32R)
            nc.vector.tensor_copy(u_r[:], u_t[:])
            nc.vector.tensor_copy(j_r[:], j_t[:])

            from contextlib import ExitStack
            loop_ctx = ExitStack()
            if loop_n > 1:
                loop_ctx.enter_context(tc.For_i(0, loop_n, 1))
            H = G // 2          # half-wave store granularity
            for _ in range(n_iter):
                W = None            # exact f32 sum of all completed waves
                s_prev = None       # exact f32 within-wave prefix s_{j-1}
                carry = None        # F32R carry operand(s) for next block
                for w in range(NW):
                    xw = xin.tile([P, G, C], F32, tag="xw")
                    xv = x_d[w * G * P:(w + 1) * G * P, :].rearrange(
                        "(j p) c -> p j c", p=P)
                    in_eng = (nc.scalar if in_ring == "alt" and w % 2
                              else nc.sync)
                    if w == 0 and loop_n == 1:
                        # split the first load so PE starts sooner on a cold
                        # start; in the steady-state loop it is pure overhead
                        for q in range(G // 2):
                            nc.sync.dma_start(xw[:, 2 * q:2 * q + 2, :],
                                              xv[:, 2 * q:2 * q + 2, :])
                    else:
                        in_eng.dma_start(xw[:], xv)
                    ow = oput.tile([P, G, C], OUT_DT, tag="ow")
                    xr_view = {}
                    for j in range(G):
                        k = w * G + j
                        xk = xw[:, j, :]
                        ps = psp.tile([P, C], F32, tag="ps")
                        if k == 0:
                            # Block 0 divides by small counts (1..128), so a
                            # single fp32r rounding of x would blow the
                            # rel-err floor there.  Its free carry-matmul
                            # slot pays for an exact hi/lo split instead:
                            # hi == round_f32r(x_0).
                            z0 = zp.tile([P, C], F32R, tag="z")
                            nc.vector.tensor_copy(z0[:], xk)
                            xl = xrp.tile([P, C], F32R, tag="xr")
                            nc.vector.tensor_sub(xl[:], xk,
                                                 z0[:].bitcast(F32))
                            nc.tensor.matmul(ps[:], u_r[:], z0[:],
                                             start=True, stop=False)
                            nc.tensor.matmul(ps[:], u_r[:], xl[:],
                                             start=False, stop=True)
                            carry = (z0, xl)       # z_0 hi/lo, free reuse
                        else:
                            if pair_cast:
                                if j in xr_view:
                                    xr_ap = xr_view.pop(j)
                                else:
                                    span = min(2, G - j)
                                    xrt = xrp.tile([P, span, C], F32R,
                                                   tag="xr")
                                    cast_r(xrt[:], xw[:, j:j + span, :])
                                    xr_ap = xrt[:, 0, :]
                                    if span == 2:
                                        xr_view[j + 1] = xrt[:, 1, :]
                            else:
                                xr = xrp.tile([P, C], F32R, tag="xr")
                                cast_r(xr[:], xk)
                                xr_ap = xr[:]
                            nc.tensor.matmul(ps[:], u_r[:], xr_ap,
                                             start=True, stop=False)
                            if len(carry) == 2:
                                zh, zl = carry
                                nc.tensor.matmul(ps[:], j_r[:], zh[:],
                                                 start=False, stop=False)
                                nc.tensor.matmul(ps[:], j_r[:], zl[:],
                                                 start=False, stop=True)
                            else:
                                nc.tensor.matmul(ps[:], j_r[:], carry[0][:],
                                                 start=False, stop=True)
                        # Two-level exact prefix state.  s_j = x_{w,0..j}
                        # (f32, exact); W = sum of all completed waves (f32,
                        # exact).  Each block's carry operand is rounded to
                        # f32r ONCE from exact f32 operands — a single-ulp
                        # error per use, instead of the biased random walk a
                        # 63-step rounded chain accumulates.
                        if j == 0:
                            s_j = xk               # s_0 view; no op
                        else:
                            s_new = sp.tile([P, C], F32, tag="s")
                            nc.vector.tensor_add(s_new[:], s_prev, xk)
                            s_j = s_new[:]
                        s_prev = s_j
                        if k == NB - 1:
                            pass                   # no more carries needed
                        elif k == 0:
                            pass                   # carry (z0, xl) already set
                        elif W is None:
                            # wave 0: counts are still small enough that a
                            # single f32r rounding of the carry breaks the
                            # rel-err floor — split s_j exactly into hi+lo
                            # (one extra carry matmul for these 7 blocks)
                            sh = zp.tile([P, C], F32R, tag="z")
                            cast_r(sh[:], s_j)
                            sl = xrp.tile([P, C], F32R, tag="xr")
                            nc.vector.tensor_sub(sl[:], s_j,
                                                 sh[:].bitcast(F32))
                            carry = (sh, sl)
                            if j == G - 1:
                                W = s_j            # exact wave-0 sum (view)
                        elif j < G - 1:
                            zr = zp.tile([P, C], F32R, tag="z")
                            zr_add(zr[:], W, s_j)
                            carry = (zr,)
                        else:
                            # wave boundary: W += s_7 (exact) and round the
                            # new W for the next wave's first carry
                            zr = zp.tile([P, C], F32R, tag="z")
                            W_new = wp.tile([P, C], F32, tag="w")
                            nc.vector.tensor_add(W_new[:], W, s_j)
                            zr_add(zr[:], W, s_j)
                            W = W_new[:]
                            carry = (zr,)
                        # evacuate PSUM with the 1/(t+1) scale, narrow to bf16
                        evac(ow[:, j, :], ps[:], r_t[:, k:k + 1])
                    ov = o_d[w * G * P:(w + 1) * G * P, :].rearrange(
                        "(j p) c -> p j c", p=P)
                    # store per half-wave so the DMA starts 4 blocks earlier;
                    # quarter the final stores (cold start only) to shorten
                    # the drain — in the loop it is steady-state overhead
                    if w == NW - 1 and loop_n == 1:
                        for q in range(G // 2):
                            eng = (nc.sync if store_mode == "alt" and q % 2
                                   else nc.scalar)
                            eng.dma_start(ov[:, 2 * q:2 * q + 2, :],
                                          ow[:, 2 * q:2 * q + 2, :])
                    elif store_mode == "wave":
                        nc.scalar.dma_start(ov[:], ow[:])
                    elif store_mode == "alt":
                        nc.scalar.dma_start(ov[:, :H, :], ow[:, :H, :])
                        nc.sync.dma_start(ov[:, H:, :], ow[:, H:, :])
                    else:
                        nc.scalar.dma_start(ov[:, :H, :], ow[:, :H, :])
                        nc.scalar.dma_start(ov[:, H:, :], ow[:, H:, :])
            loop_ctx.close()

    nc.compile()
    return nc


def make_consts():
    s = np.arange(P)
    u = (s[:, None] <= s[None, :]).astype(np.float32)          # u[s,t]=1 if s<=t
    jm = np.ones((P, P), dtype=np.float32)
    counts = (np.arange(NB)[None, :] * P + s[:, None] + 1)     # [P, NB]
    recip = (1.0 / counts).astype(np.float32)
    return u, jm, recip


def kernel(x):
    x = np.ascontiguousarray(np.asarray(x), dtype=np.float32)
    assert x.shape == (B, T, C), x.shape
    if "nc" not in _cache:
        _cache["nc"] = build_program()
    nc = _cache["nc"]
    u, jm, recip = make_consts()
    in_maps = [{"x": x[b], "u": u, "jm": jm, "recip": recip}
               for b in range(N_CORES)]
    res = run_bass_kernel_spmd(nc, in_maps, list(range(N_CORES)))
    out = np.stack([np.asarray(res.results[b]["out"], dtype=np.float32)
                    for b in range(N_CORES)], axis=0)
    return out



# revision 18
# speedup vs baseline: 1.2031x; 1.2031x over previous
"""Causal bag-of-words kernel for Trainium2 (8 NeuronCores, SPMD).

out[b, t, :] = mean(x[b, :t+1, :], axis=0)  for x of shape (8, 8192, 512) f32.

Sharding: data-parallel over B — core b handles x[b] (8192, 512) independently.

Per-core algorithm (natural [t, c] layout, no transposes):
  T = 8192 is split into 64 blocks of 128 rows (partition dim).
  For block k with rows X_k [128, 512]:
    psum_k = U @ Xr_k + J @ Zr_{k-1}     (two full-rate fp32r PE matmuls)
  where U is upper-triangular ones (cumsum within the block), J is all-ones
  (broadcasts the column-sum of Z over all 128 rows), Xr_k is X_k rounded to
  fp32r (one DVE copy), and Zr_{k-1} rounds the exact f32 running sum of
  blocks 0..k-1.  fp32r (TF32-like) runs 1 cycle/row at N=512 vs 4 for ieee
  fp32 — trading ~2^-12 relative error for a 4x PE-time cut (tolerance is
  2e-2).  Precision structure:
   - The running sum is kept EXACT in f32 as a two-level prefix (within-wave
     prefixes s_j on a 7-add serial DVE chain per wave, plus a once-per-wave
     wave-total W), and each carry operand Zr is rounded ONCE from exact
     operands — avoiding the biased ~2^-12*sqrt(k) random walk a rounded
     63-step chain accumulates.
   - Block 0 (counts 1..128) and the carries of wave 0 (counts < 1024) are
     computed exactly via hi/lo fp32r splits, reusing the free carry-matmul
     slot of block 0 and one extra carry matmul for blocks 1..8.
  The 1/(t+1) scaling is folded into the PSUM->SBUF evacuation (ACT
  activation with a per-partition scale vector), which also narrows the
  output to bf16 — halving store-side HBM traffic (the kernel is
  memory-bound; ~2^-9 output rounding is far inside the tolerance).  The
  host upcasts back to f32.
  Blocks stream in waves of 8 (2 MiB input DMAs on the SP HWDGE ring);
  stores go per half-wave on the ACT HWDGE ring.  Elementwise work stays on
  DVE (+ACT for evacuation): GPSIMD tensor ops and ACT casts measured 1.5-2x
  slower on real hardware despite favorable cost-model predictions.
"""

import sys

sys.path.insert(0, "/opt/trn_rl_repo")

import numpy as np

import concourse.bacc as bacc
import concourse.bass as bass
import concourse.mybir as mybir
import concourse.tile as tile
from concourse.bass_utils import run_bass_kernel_spmd

B, T, C = 8, 8192, 512
P = 128                 # partition dim / block size along T
NB = T // P             # 64 blocks
G = 8                   # blocks per wave (2 MiB per input DMA)
NW = NB // G            # 8 waves
N_CORES = 8
F32 = mybir.dt.float32
F32R = mybir.dt.float32r  # full-rate fp32 matmul path (4x faster at N>=256)
BF16 = mybir.dt.bfloat16

_cache: dict = {}


def build_program(algo: str = "v2", **kw):
    if algo == "v2":
        return _build_v2(**kw)
    return _build_v1(**kw)


def _build_v1(n_iter: int = 1, loop_n: int = 1, g: int = 8,
              out_dt: str = "bf16", cast_engine: str = "vector",
              zdt: str = "f32r", evac_engine: str = "scalar",
              pair_cast: bool = False, store_mode: str = "half",
              in_ring: str = "sync", zr_engine: str = "vector",
              xin_bufs: int = 4, obufs: int = 3, zbufs: int = 8,
              xrbufs: int = 8, psbufs: int = 8):
    """Build + compile the per-core Bass program (SPMD, identical on all cores).

    n_iter > 1 unrolls the whole computation; loop_n > 1 wraps it in a
    hardware For_i loop (both for timing by the slope method); results are
    identical for any value.
    cast_engine: engine that pre-rounds x to fp32r (the BIR verifier requires
    every fp32r-matmul operand's producer to write rounded F32R).
    zdt: dtype of the running block-sum chain ('f32r' feeds the carry matmul
    directly; 'f32' is invalid for the fp32r path).
    out_dt: 'bf16' (half store traffic; host upcasts) or 'f32'.
    """
    G = g
    NW = NB // G
    nc = bacc.Bacc("TRN2", target_bir_lowering=False, debug=False,
                   num_devices=N_CORES)
    OUT_DT = BF16 if out_dt == "bf16" else F32
    ZDT = F32R if zdt == "f32r" else F32

    x_d = nc.dram_tensor("x", [T, C], F32, kind="ExternalInput")
    u_d = nc.dram_tensor("u", [P, P], F32, kind="ExternalInput")
    j_d = nc.dram_tensor("jm", [P, P], F32, kind="ExternalInput")
    r_d = nc.dram_tensor("recip", [P, NB], F32, kind="ExternalInput")
    o_d = nc.dram_tensor("out", [T, C], OUT_DT, kind="ExternalOutput")

    ACT_COPY = mybir.ActivationFunctionType.Copy
    cast_cycle = cast_engine.split(",")
    evac_cycle = evac_engine.split(",")
    zr_cycle = zr_engine.split(",")
    cnt = {"cast": 0, "evac": 0, "zr": 0}

    def cast_r(dst, src):
        eng = cast_cycle[cnt["cast"] % len(cast_cycle)]
        cnt["cast"] += 1
        if eng == "scalar":
            nc.scalar.activation(dst, src, ACT_COPY)
        else:
            getattr(nc, eng).tensor_copy(dst, src)

    def zr_add(dst, a, b):
        eng = zr_cycle[cnt["zr"] % len(zr_cycle)]
        cnt["zr"] += 1
        getattr(nc, eng).tensor_add(dst, a, b)

    def evac(dst, ps_ap, r_ap):
        eng = evac_cycle[cnt["evac"] % len(evac_cycle)]
        cnt["evac"] += 1
        if eng == "scalar":
            nc.scalar.activation(dst, ps_ap, ACT_COPY, scale=r_ap)
        else:
            getattr(nc, eng).tensor_scalar_mul(dst, ps_ap, r_ap)

    with tile.TileContext(nc) as tc:
        with (
            tc.tile_pool(name="consts", bufs=1) as consts,
            tc.tile_pool(name="xin", bufs=xin_bufs) as xin,
            tc.tile_pool(name="oput", bufs=obufs) as oput,
            tc.tile_pool(name="zp", bufs=zbufs) as zp,
            tc.tile_pool(name="xr", bufs=xrbufs) as xrp,
            tc.tile_pool(name="sp", bufs=12) as sp,
            tc.tile_pool(name="wp", bufs=3) as wp,
            tc.tile_pool(name="ps", bufs=psbufs, space="PSUM") as psp,
        ):
            # consts go via SWDGE (gpsimd) so the HWDGE rings start on the
            # first wave load immediately
            u_t = consts.tile([P, P], F32)
            j_t = consts.tile([P, P], F32)
            r_t = consts.tile([P, NB], F32)
            nc.gpsimd.dma_start(u_t[:], u_d[:])
            nc.gpsimd.dma_start(j_t[:], j_d[:])
            nc.gpsimd.dma_start(r_t[:], r_d[:])
            # 0/1 matrices are exact in fp32r
            u_r = consts.tile([P, P], F32R)
            j_r = consts.tile([P, P], F32R)
            nc.vector.tensor_copy(u_r[:], u_t[:])
            nc.vector.tensor_copy(j_r[:], j_t[:])

            from contextlib import ExitStack
            loop_ctx = ExitStack()
            if loop_n > 1:
                loop_ctx.enter_context(tc.For_i(0, loop_n, 1))
            H = G // 2          # half-wave store granularity
            for _ in range(n_iter):
                W = None            # exact f32 sum of all completed waves
                s_prev = None       # exact f32 within-wave prefix s_{j-1}
                carry = None        # F32R carry operand(s) for next block
                for w in range(NW):
                    xw = xin.tile([P, G, C], F32, tag="xw")
                    xv = x_d[w * G * P:(w + 1) * G * P, :].rearrange(
                        "(j p) c -> p j c", p=P)
                    in_eng = (nc.scalar if in_ring == "alt" and w % 2
                              else nc.sync)
                    if w == 0 and loop_n == 1:
                        # split the first load so PE starts sooner on a cold
                        # start; in the steady-state loop it is pure overhead
                        for q in range(G // 2):
                            nc.sync.dma_start(xw[:, 2 * q:2 * q + 2, :],
                                              xv[:, 2 * q:2 * q + 2, :])
                    else:
                        in_eng.dma_start(xw[:], xv)
                    ow = oput.tile([P, G, C], OUT_DT, tag="ow")
                    xr_view = {}
                    for j in range(G):
                        k = w * G + j
                        xk = xw[:, j, :]
                        ps = psp.tile([P, C], F32, tag="ps")
                        if k == 0:
                            # Block 0 divides by small counts (1..128), so a
                            # single fp32r rounding of x would blow the
                            # rel-err floor there.  Its free carry-matmul
                            # slot pays for an exact hi/lo split instead:
                            # hi == round_f32r(x_0).
                            z0 = zp.tile([P, C], F32R, tag="z")
                            nc.vector.tensor_copy(z0[:], xk)
                            xl = xrp.tile([P, C], F32R, tag="xr")
                            nc.vector.tensor_sub(xl[:], xk,
                                                 z0[:].bitcast(F32))
                            nc.tensor.matmul(ps[:], u_r[:], z0[:],
                                             start=True, stop=False)
                            nc.tensor.matmul(ps[:], u_r[:], xl[:],
                                             start=False, stop=True)
                            carry = (z0, xl)       # z_0 hi/lo, free reuse
                        else:
                            if pair_cast:
                                if j in xr_view:
                                    xr_ap = xr_view.pop(j)
                                else:
                                    span = min(2, G - j)
                                    xrt = xrp.tile([P, span, C], F32R,
                                                   tag="xr")
                                    cast_r(xrt[:], xw[:, j:j + span, :])
                                    xr_ap = xrt[:, 0, :]
                                    if span == 2:
                                        xr_view[j + 1] = xrt[:, 1, :]
                            else:
                                xr = xrp.tile([P, C], F32R, tag="xr")
                                cast_r(xr[:], xk)
                                xr_ap = xr[:]
                            nc.tensor.matmul(ps[:], u_r[:], xr_ap,
                                             start=True, stop=False)
                            if len(carry) == 2:
                                zh, zl = carry
                                nc.tensor.matmul(ps[:], j_r[:], zh[:],
                                                 start=False, stop=False)
                                nc.tensor.matmul(ps[:], j_r[:], zl[:],
                                                 start=False, stop=True)
                            else:
                                nc.tensor.matmul(ps[:], j_r[:], carry[0][:],
                                                 start=False, stop=True)
                        # Two-level exact prefix state.  s_j = x_{w,0..j}
                        # (f32, exact); W = sum of all completed waves (f32,
                        # exact).  Each block's carry operand is rounded to
                        # f32r ONCE from exact f32 operands — a single-ulp
                        # error per use, instead of the biased random walk a
                        # 63-step rounded chain accumulates.
                        if j == 0:
                            s_j = xk               # s_0 view; no op
                        else:
                            s_new = sp.tile([P, C], F32, tag="s")
                            nc.vector.tensor_add(s_new[:], s_prev, xk)
                            s_j = s_new[:]
                        s_prev = s_j
                        if k == NB - 1:
                            pass                   # no more carries needed
                        elif k == 0:
                            pass                   # carry (z0, xl) already set
                        elif W is None:
                            # wave 0: counts are still small enough that a
                            # single f32r rounding of the carry breaks the
                            # rel-err floor — split s_j exactly into hi+lo
                            # (one extra carry matmul for these 7 blocks)
                            sh = zp.tile([P, C], F32R, tag="z")
                            cast_r(sh[:], s_j)
                            sl = xrp.tile([P, C], F32R, tag="xr")
                            nc.vector.tensor_sub(sl[:], s_j,
                                                 sh[:].bitcast(F32))
                            carry = (sh, sl)
                            if j == G - 1:
                                W = s_j            # exact wave-0 sum (view)
                        elif j < G - 1:
                            zr = zp.tile([P, C], F32R, tag="z")
                            zr_add(zr[:], W, s_j)
                            carry = (zr,)
                        else:
                            # wave boundary: W += s_7 (exact) and round the
                            # new W for the next wave's first carry
                            zr = zp.tile([P, C], F32R, tag="z")
                            W_new = wp.tile([P, C], F32, tag="w")
                            nc.vector.tensor_add(W_new[:], W, s_j)
                            zr_add(zr[:], W, s_j)
                            W = W_new[:]
                            carry = (zr,)
                        # evacuate PSUM with the 1/(t+1) scale, narrow to bf16
                        evac(ow[:, j, :], ps[:], r_t[:, k:k + 1])
                    ov = o_d[w * G * P:(w + 1) * G * P, :].rearrange(
                        "(j p) c -> p j c", p=P)
                    # store per half-wave so the DMA starts 4 blocks earlier;
                    # quarter the final stores (cold start only) to shorten
                    # the drain — in the loop it is steady-state overhead
                    if w == NW - 1 and loop_n == 1:
                        for q in range(G // 2):
                            eng = (nc.sync if store_mode == "alt" and q % 2
                                   else nc.scalar)
                            eng.dma_start(ov[:, 2 * q:2 * q + 2, :],
                                          ow[:, 2 * q:2 * q + 2, :])
                    elif store_mode == "wave":
                        nc.scalar.dma_start(ov[:], ow[:])
                    elif store_mode == "alt":
                        nc.scalar.dma_start(ov[:, :H, :], ow[:, :H, :])
                        nc.sync.dma_start(ov[:, H:, :], ow[:, H:, :])
                    else:
                        nc.scalar.dma_start(ov[:, :H, :], ow[:, :H, :])
                        nc.scalar.dma_start(ov[:, H:, :], ow[:, H:, :])
            loop_ctx.close()

    nc.compile()
    return nc


def _build_v2(n_iter: int = 1, loop_n: int = 1, g: int = 8,
              out_dt: str = "bf16", evac_engine: str = "scalar",
              store_mode: str = "half", in_ring: str = "sync",
              s_mode: str = "round", round_engine: str = "gpsimd",
              w_engine: str = "vector", exact0: bool = True,
              s0_split: bool = True, w_split: bool = True,
              xin_bufs: int = 4, obufs: int = 3, sbufs: int = 4,
              srbufs: int = 4, wbufs: int = 3, psbufs: int = 8):
    """v2: carries come from the PE instead of a DVE-materialized operand.

    x is DMA'd straight into F32R tiles (the DRAM tensor is declared f32r, so
    no rounding op exists anywhere on the x path — the PE rounds internally;
    the BIR verifier accepts DMA as an F32R producer).  Per block k = (w, j):
        psum_k = J @ W_r[w]  +  J @ s_r[j-1]  +  U @ X_k      (all fp32r)
    where W_r[w] is the once-per-wave rounded total of completed waves and
    s_r[j-1] is a one-shot F32R rounding (round_engine, default gpsimd — the
    Pool engine is otherwise idle) of the exact f32 within-wave prefix chain
    kept on DVE.  DVE work per block drops to ONE f32 add; the old per-block
    zr_add and pre-cast disappear.  f32r hardware rounding is coarse
    (~bf16-level, truncating): s_mode='f32r' (chain writes F32R directly)
    compounds that rounding ~7x per wave and measured 5.5e-2 rel err — every
    F32R operand must be rounded ONCE from exact f32 values (v1's invariant).
    """
    G = g
    NW = NB // G
    nc = bacc.Bacc("TRN2", target_bir_lowering=False, debug=False,
                   num_devices=N_CORES)
    OUT_DT = BF16 if out_dt == "bf16" else F32

    x_d = nc.dram_tensor("x", [T, C], F32R, kind="ExternalInput")
    u_d = nc.dram_tensor("u", [P, P], F32, kind="ExternalInput")
    j_d = nc.dram_tensor("jm", [P, P], F32, kind="ExternalInput")
    r_d = nc.dram_tensor("recip", [P, NB], F32, kind="ExternalInput")
    o_d = nc.dram_tensor("out", [T, C], OUT_DT, kind="ExternalOutput")
    if exact0:
        # F32R inputs arrive 12-bit rounded, so an exact hi/lo split of
        # block 0 needs its rows again as true f32 (+256 KB, ~1% traffic)
        x0_d = nc.dram_tensor("x0", [P, C], F32, kind="ExternalInput")

    ACT_COPY = mybir.ActivationFunctionType.Copy
    evac_cycle = evac_engine.split(",")
    w_cycle = w_engine.split(",")
    cnt = {"evac": 0, "w": 0}

    def evac(dst, ps_ap, r_ap):
        eng = evac_cycle[cnt["evac"] % len(evac_cycle)]
        cnt["evac"] += 1
        if eng == "scalar":
            nc.scalar.activation(dst, ps_ap, ACT_COPY, scale=r_ap)
        else:
            getattr(nc, eng).tensor_scalar_mul(dst, ps_ap, r_ap)

    def w_op(op, dst, *srcs):
        eng = w_cycle[cnt["w"] % len(w_cycle)]
        cnt["w"] += 1
        if op == "add":
            getattr(nc, eng).tensor_add(dst, *srcs)
        elif eng == "scalar":
            nc.scalar.activation(dst, srcs[0], ACT_COPY)
        else:
            getattr(nc, eng).tensor_copy(dst, srcs[0])

    round_cycle = round_engine.split(",")

    def round_op(dst, src):
        eng = round_cycle[cnt.setdefault("r", 0) % len(round_cycle)]
        cnt["r"] += 1
        if eng == "scalar":
            nc.scalar.activation(dst, src, ACT_COPY)
        else:
            getattr(nc, eng).tensor_copy(dst, src)

    S_DT = F32R if s_mode == "f32r" else F32

    with tile.TileContext(nc) as tc:
        with (
            tc.tile_pool(name="consts", bufs=1) as consts,
            tc.tile_pool(name="xin", bufs=xin_bufs) as xin,
            tc.tile_pool(name="oput", bufs=obufs) as oput,
            tc.tile_pool(name="sp", bufs=sbufs) as sp,
            tc.tile_pool(name="srp", bufs=max(srbufs, 6)) as srp,
            tc.tile_pool(name="wp", bufs=wbufs) as wp,
            tc.tile_pool(name="wrp", bufs=wbufs) as wrp,
            tc.tile_pool(name="wlp", bufs=wbufs) as wlp,
            tc.tile_pool(name="ps", bufs=psbufs, space="PSUM") as psp,
        ):
            u_t = consts.tile([P, P], F32)
            j_t = consts.tile([P, P], F32)
            r_t = consts.tile([P, NB], F32)
            nc.gpsimd.dma_start(u_t[:], u_d[:])
            nc.gpsimd.dma_start(j_t[:], j_d[:])
            nc.gpsimd.dma_start(r_t[:], r_d[:])
            u_r = consts.tile([P, P], F32R)
            j_r = consts.tile([P, P], F32R)
            nc.vector.tensor_copy(u_r[:], u_t[:])
            nc.vector.tensor_copy(j_r[:], j_t[:])

            from contextlib import ExitStack
            loop_ctx = ExitStack()
            if loop_n > 1:
                loop_ctx.enter_context(tc.For_i(0, loop_n, 1))
            H = G // 2
            for _ in range(n_iter):
                W_f = None          # exact f32 sum of completed waves (AP)
                W_ops = []          # F32R carry operand(s) for J-matmuls
                for w in range(NW):
                    if w == 0 and exact0:
                        # block-0 exact path: true-f32 rows first on the ring
                        xf_t = sp.tile([P, C], F32, tag="s")
                        nc.sync.dma_start(xf_t[:], x0_d[:])
                        z0 = srp.tile([P, C], F32R, tag="sr")
                        nc.vector.tensor_copy(z0[:], xf_t[:])
                        xl = srp.tile([P, C], F32R, tag="sr")
                        nc.vector.tensor_sub(xl[:], xf_t[:],
                                             z0[:].bitcast(F32))
                    xw = xin.tile([P, G, C], F32R, tag="xw")
                    xv = x_d[w * G * P:(w + 1) * G * P, :].rearrange(
                        "(j p) c -> p j c", p=P)
                    in_eng = (nc.scalar if in_ring == "alt" and w % 2
                              else nc.sync)
                    if w == 0 and loop_n == 1:
                        for q in range(G // 2):
                            nc.sync.dma_start(xw[:, 2 * q:2 * q + 2, :],
                                              xv[:, 2 * q:2 * q + 2, :])
                    else:
                        in_eng.dma_start(xw[:], xv)
                    ow = oput.tile([P, G, C], OUT_DT, tag="ow")
                    s_prev = None     # exact f32 within-wave prefix s_{j-1}
                    s_ops = []        # F32R operand(s) carrying s_{j-1}
                    for j in range(G):
                        k = w * G + j
                        xk_r = xw[:, j, :]
                        xk_f = xw[:, j, :].bitcast(F32)
                        ps = psp.tile([P, C], F32, tag="ps")
                        # Block 0 divides by tiny counts (1..128) where the
                        # metric's denom floor bites, so a 12-bit rounding of
                        # x is too coarse — split the true-f32 x_0 exactly
                        # into hi+lo (the f32 sub is exact) and spend one
                        # extra U-matmul.  (z0, xl) then double as an EXACT
                        # carry for block 1.
                        if k == 0 and exact0:
                            u_ops = [z0[:], xl[:]]
                            xk_f = xf_t[:]   # exact f32 x_0 for the s chain
                        else:
                            u_ops = [xk_r]
                        # carry matmuls first so the PE needn't wait on the
                        # wave's X DMA to start the group
                        carry_ops = list(W_ops) + list(s_ops)
                        n_mm = len(carry_ops) + len(u_ops)
                        mm_i = 0

                        def mm(lhs, rhs):
                            nonlocal mm_i
                            nc.tensor.matmul(ps[:], lhs, rhs,
                                             start=(mm_i == 0),
                                             stop=(mm_i == n_mm - 1))
                            mm_i += 1

                        for op in carry_ops:
                            mm(j_r[:], op)
                        for op in u_ops:
                            mm(u_r[:], op)
                        # exact within-wave prefix chain (ONE DVE add per
                        # block); carry operand = one-shot rounding on the
                        # otherwise-idle round_engine.  s_0 is the raw x view
                        # (the PE rounds it internally, once).
                        if j == 0:
                            s_prev = xk_f
                            s_ops = u_ops
                        elif k == NB - 1:
                            pass             # nothing consumes s_63
                        else:
                            s_t = sp.tile([P, C], F32, tag="s")
                            nc.vector.tensor_add(s_t[:], s_prev, xk_f)
                            s_prev = s_t[:]
                            if j < G - 1:   # s_{G-1} only feeds W (f32)
                                sr_t = srp.tile([P, C], F32R, tag="sr")
                                round_op(sr_t[:], s_t[:])
                                if w == 0 and s0_split:
                                    # wave 0: carry ≈ whole prefix, so keep
                                    # it exact via hi+lo (extra J-matmul)
                                    sl_t = srp.tile([P, C], F32R, tag="sr")
                                    nc.vector.tensor_sub(
                                        sl_t[:], s_t[:],
                                        sr_t[:].bitcast(F32))
                                    s_ops = [sr_t[:], sl_t[:]]
                                else:
                                    s_ops = [sr_t[:]]
                        # wave boundary: fold the wave into W (exact f32 add)
                        # and round once for the next wave's carry matmuls;
                        # w_split spends an extra J-matmul to keep the W part
                        # of every later carry exact (cancellation in late
                        # csums amplifies carry rounding noise)
                        if j == G - 1 and w < NW - 1:
                            if W_f is None:
                                W_f = s_prev
                            else:
                                W_t = wp.tile([P, C], F32, tag="w")
                                w_op("add", W_t[:], W_f, s_prev)
                                W_f = W_t[:]
                            Wr_t = wrp.tile([P, C], F32R, tag="wr")
                            w_op("copy", Wr_t[:], W_f)
                            if w_split:
                                Wl_t = wlp.tile([P, C], F32R, tag="wl")
                                nc.vector.tensor_sub(Wl_t[:], W_f,
                                                     Wr_t[:].bitcast(F32))
                                W_ops = [Wr_t[:], Wl_t[:]]
                            else:
                                W_ops = [Wr_t[:]]
                        evac(ow[:, j, :], ps[:], r_t[:, k:k + 1])
                    ov = o_d[w * G * P:(w + 1) * G * P, :].rearrange(
                        "(j p) c -> p j c", p=P)
                    if w == NW - 1 and loop_n == 1:
                        for q in range(G // 2):
                            eng = (nc.sync if store_mode == "alt" and q % 2
                                   else nc.scalar)
                            eng.dma_start(ov[:, 2 * q:2 * q + 2, :],
                                          ow[:, 2 * q:2 * q + 2, :])
                    elif store_mode == "wave":
                        nc.scalar.dma_start(ov[:], ow[:])
                    elif store_mode == "alt":
                        nc.scalar.dma_start(ov[:, :H, :], ow[:, :H, :])
                        nc.sync.dma_start(ov[:, H:, :], ow[:, H:, :])
                    else:
                        nc.scalar.dma_start(ov[:, :H, :], ow[:, :H, :])
                        nc.scalar.dma_start(ov[:, H:, :], ow[:, H:, :])
            loop_ctx.close()

    nc.compile()
    return nc


def make_consts():
    s = np.arange(P)
    u = (s[:, None] <= s[None, :]).astype(np.float32)          # u[s,t]=1 if s<=t
    jm = np.ones((P, P), dtype=np.float32)
    counts = (np.arange(NB)[None, :] * P + s[:, None] + 1)     # [P, NB]
    recip = (1.0 / counts).astype(np.float32)
    return u, jm, recip


def program_input_names(nc):
    import concourse.mybir as _mb
    names = set()
    for alloc in nc.m.functions[0].allocations:
        if (isinstance(alloc, _mb.MemoryLocationSet)
                and alloc.kind == "ExternalInput"):
            names.add(alloc.memorylocations[0].name)
    return names


def make_in_maps(nc, x):
    """Per-core input dicts for run_bass_kernel_spmd, matching nc's inputs."""
    u, jm, recip = make_consts()
    names = program_input_names(nc)
    maps = []
    for b in range(N_CORES):
        m = {"x": x[b], "u": u, "jm": jm, "recip": recip}
        if "x0" in names:
            m["x0"] = np.ascontiguousarray(x[b][:P])
        maps.append(m)
    return maps


def kernel(x):
    x = np.ascontiguousarray(np.asarray(x), dtype=np.float32)
    assert x.shape == (B, T, C), x.shape
    if "nc" not in _cache:
        _cache["nc"] = build_program()
    nc = _cache["nc"]
    res = run_bass_kernel_spmd(nc, make_in_maps(nc, x),
                               list(range(N_CORES)))
    out = np.stack([np.asarray(res.results[b]["out"], dtype=np.float32)
                    for b in range(N_CORES)], axis=0)
    return out



# revision 19
# speedup vs baseline: 1.3012x; 1.0815x over previous
"""Causal bag-of-words kernel for Trainium2 (8 NeuronCores, SPMD).

out[b, t, :] = mean(x[b, :t+1, :], axis=0)  for x of shape (8, 8192, 512) f32.

Sharding: data-parallel over B — core b handles x[b] (8192, 512) independently.

Per-core algorithm (natural [t, c] layout, no transposes):
  T = 8192 is split into 64 blocks of 128 rows (partition dim).
  For block k with rows X_k [128, 512]:
    psum_k = U @ Xr_k + J @ Zr_{k-1}     (two full-rate fp32r PE matmuls)
  where U is upper-triangular ones (cumsum within the block), J is all-ones
  (broadcasts the column-sum of Z over all 128 rows), Xr_k is X_k rounded to
  fp32r (one DVE copy), and Zr_{k-1} rounds the exact f32 running sum of
  blocks 0..k-1.  fp32r (TF32-like) runs 1 cycle/row at N=512 vs 4 for ieee
  fp32 — trading ~2^-12 relative error for a 4x PE-time cut (tolerance is
  2e-2).  Precision structure:
   - The running sum is kept EXACT in f32 as a two-level prefix (within-wave
     prefixes s_j on a 7-add serial DVE chain per wave, plus a once-per-wave
     wave-total W), and each carry operand Zr is rounded ONCE from exact
     operands — avoiding the biased ~2^-12*sqrt(k) random walk a rounded
     63-step chain accumulates.
   - Block 0 (counts 1..128) and the carries of wave 0 (counts < 1024) are
     computed exactly via hi/lo fp32r splits, reusing the free carry-matmul
     slot of block 0 and one extra carry matmul for blocks 1..8.
  The 1/(t+1) scaling is folded into the PSUM->SBUF evacuation (ACT
  activation with a per-partition scale vector), which also narrows the
  output to bf16 — halving store-side HBM traffic (the kernel is
  memory-bound; ~2^-9 output rounding is far inside the tolerance).  The
  host upcasts back to f32.
  Blocks stream in waves of 8 (2 MiB input DMAs on the SP HWDGE ring);
  stores go per half-wave on the ACT HWDGE ring.  Elementwise work stays on
  DVE (+ACT for evacuation): GPSIMD tensor ops and ACT casts measured 1.5-2x
  slower on real hardware despite favorable cost-model predictions.
"""

import sys

sys.path.insert(0, "/opt/trn_rl_repo")

import numpy as np

import concourse.bacc as bacc
import concourse.bass as bass
import concourse.mybir as mybir
import concourse.tile as tile
from concourse.bass_utils import run_bass_kernel_spmd

B, T, C = 8, 8192, 512
P = 128                 # partition dim / block size along T
NB = T // P             # 64 blocks
G = 8                   # blocks per wave (2 MiB per input DMA)
NW = NB // G            # 8 waves
N_CORES = 8
F32 = mybir.dt.float32
F32R = mybir.dt.float32r  # full-rate fp32 matmul path (4x faster at N>=256)
BF16 = mybir.dt.bfloat16

_cache: dict = {}


def build_program(algo: str = "v2", **kw):
    if algo == "v2":
        return _build_v2(**kw)
    return _build_v1(**kw)


def _build_v1(n_iter: int = 1, loop_n: int = 1, g: int = 8,
              out_dt: str = "bf16", cast_engine: str = "vector",
              zdt: str = "f32r", evac_engine: str = "scalar",
              pair_cast: bool = False, store_mode: str = "half",
              in_ring: str = "sync", zr_engine: str = "vector",
              xin_bufs: int = 4, obufs: int = 3, zbufs: int = 8,
              xrbufs: int = 8, psbufs: int = 8):
    """Build + compile the per-core Bass program (SPMD, identical on all cores).

    n_iter > 1 unrolls the whole computation; loop_n > 1 wraps it in a
    hardware For_i loop (both for timing by the slope method); results are
    identical for any value.
    cast_engine: engine that pre-rounds x to fp32r (the BIR verifier requires
    every fp32r-matmul operand's producer to write rounded F32R).
    zdt: dtype of the running block-sum chain ('f32r' feeds the carry matmul
    directly; 'f32' is invalid for the fp32r path).
    out_dt: 'bf16' (half store traffic; host upcasts) or 'f32'.
    """
    G = g
    NW = NB // G
    nc = bacc.Bacc("TRN2", target_bir_lowering=False, debug=False,
                   num_devices=N_CORES)
    OUT_DT = BF16 if out_dt == "bf16" else F32
    ZDT = F32R if zdt == "f32r" else F32

    x_d = nc.dram_tensor("x", [T, C], F32, kind="ExternalInput")
    u_d = nc.dram_tensor("u", [P, P], F32, kind="ExternalInput")
    j_d = nc.dram_tensor("jm", [P, P], F32, kind="ExternalInput")
    r_d = nc.dram_tensor("recip", [P, NB], F32, kind="ExternalInput")
    o_d = nc.dram_tensor("out", [T, C], OUT_DT, kind="ExternalOutput")

    ACT_COPY = mybir.ActivationFunctionType.Copy
    cast_cycle = cast_engine.split(",")
    evac_cycle = evac_engine.split(",")
    zr_cycle = zr_engine.split(",")
    cnt = {"cast": 0, "evac": 0, "zr": 0}

    def cast_r(dst, src):
        eng = cast_cycle[cnt["cast"] % len(cast_cycle)]
        cnt["cast"] += 1
        if eng == "scalar":
            nc.scalar.activation(dst, src, ACT_COPY)
        else:
            getattr(nc, eng).tensor_copy(dst, src)

    def zr_add(dst, a, b):
        eng = zr_cycle[cnt["zr"] % len(zr_cycle)]
        cnt["zr"] += 1
        getattr(nc, eng).tensor_add(dst, a, b)

    def evac(dst, ps_ap, r_ap):
        eng = evac_cycle[cnt["evac"] % len(evac_cycle)]
        cnt["evac"] += 1
        if eng == "scalar":
            nc.scalar.activation(dst, ps_ap, ACT_COPY, scale=r_ap)
        else:
            getattr(nc, eng).tensor_scalar_mul(dst, ps_ap, r_ap)

    with tile.TileContext(nc) as tc:
        with (
            tc.tile_pool(name="consts", bufs=1) as consts,
            tc.tile_pool(name="xin", bufs=xin_bufs) as xin,
            tc.tile_pool(name="oput", bufs=obufs) as oput,
            tc.tile_pool(name="zp", bufs=zbufs) as zp,
            tc.tile_pool(name="xr", bufs=xrbufs) as xrp,
            tc.tile_pool(name="sp", bufs=12) as sp,
            tc.tile_pool(name="wp", bufs=3) as wp,
            tc.tile_pool(name="ps", bufs=psbufs, space="PSUM") as psp,
        ):
            # consts go via SWDGE (gpsimd) so the HWDGE rings start on the
            # first wave load immediately
            u_t = consts.tile([P, P], F32)
            j_t = consts.tile([P, P], F32)
            r_t = consts.tile([P, NB], F32)
            nc.gpsimd.dma_start(u_t[:], u_d[:])
            nc.gpsimd.dma_start(j_t[:], j_d[:])
            nc.gpsimd.dma_start(r_t[:], r_d[:])
            # 0/1 matrices are exact in fp32r
            u_r = consts.tile([P, P], F32R)
            j_r = consts.tile([P, P], F32R)
            nc.vector.tensor_copy(u_r[:], u_t[:])
            nc.vector.tensor_copy(j_r[:], j_t[:])

            from contextlib import ExitStack
            loop_ctx = ExitStack()
            if loop_n > 1:
                loop_ctx.enter_context(tc.For_i(0, loop_n, 1))
            H = G // 2          # half-wave store granularity
            for _ in range(n_iter):
                W = None            # exact f32 sum of all completed waves
                s_prev = None       # exact f32 within-wave prefix s_{j-1}
                carry = None        # F32R carry operand(s) for next block
                for w in range(NW):
                    xw = xin.tile([P, G, C], F32, tag="xw")
                    xv = x_d[w * G * P:(w + 1) * G * P, :].rearrange(
                        "(j p) c -> p j c", p=P)
                    in_eng = (nc.scalar if in_ring == "alt" and w % 2
                              else nc.sync)
                    if w == 0 and loop_n == 1:
                        # split the first load so PE starts sooner on a cold
                        # start; in the steady-state loop it is pure overhead
                        for q in range(G // 2):
                            nc.sync.dma_start(xw[:, 2 * q:2 * q + 2, :],
                                              xv[:, 2 * q:2 * q + 2, :])
                    else:
                        in_eng.dma_start(xw[:], xv)
                    ow = oput.tile([P, G, C], OUT_DT, tag="ow")
                    xr_view = {}
                    for j in range(G):
                        k = w * G + j
                        xk = xw[:, j, :]
                        ps = psp.tile([P, C], F32, tag="ps")
                        if k == 0:
                            # Block 0 divides by small counts (1..128), so a
                            # single fp32r rounding of x would blow the
                            # rel-err floor there.  Its free carry-matmul
                            # slot pays for an exact hi/lo split instead:
                            # hi == round_f32r(x_0).
                            z0 = zp.tile([P, C], F32R, tag="z")
                            nc.vector.tensor_copy(z0[:], xk)
                            xl = xrp.tile([P, C], F32R, tag="xr")
                            nc.vector.tensor_sub(xl[:], xk,
                                                 z0[:].bitcast(F32))
                            nc.tensor.matmul(ps[:], u_r[:], z0[:],
                                             start=True, stop=False)
                            nc.tensor.matmul(ps[:], u_r[:], xl[:],
                                             start=False, stop=True)
                            carry = (z0, xl)       # z_0 hi/lo, free reuse
                        else:
                            if pair_cast:
                                if j in xr_view:
                                    xr_ap = xr_view.pop(j)
                                else:
                                    span = min(2, G - j)
                                    xrt = xrp.tile([P, span, C], F32R,
                                                   tag="xr")
                                    cast_r(xrt[:], xw[:, j:j + span, :])
                                    xr_ap = xrt[:, 0, :]
                                    if span == 2:
                                        xr_view[j + 1] = xrt[:, 1, :]
                            else:
                                xr = xrp.tile([P, C], F32R, tag="xr")
                                cast_r(xr[:], xk)
                                xr_ap = xr[:]
                            nc.tensor.matmul(ps[:], u_r[:], xr_ap,
                                             start=True, stop=False)
                            if len(carry) == 2:
                                zh, zl = carry
                                nc.tensor.matmul(ps[:], j_r[:], zh[:],
                                                 start=False, stop=False)
                                nc.tensor.matmul(ps[:], j_r[:], zl[:],
                                                 start=False, stop=True)
                            else:
                                nc.tensor.matmul(ps[:], j_r[:], carry[0][:],
                                                 start=False, stop=True)
                        # Two-level exact prefix state.  s_j = x_{w,0..j}
                        # (f32, exact); W = sum of all completed waves (f32,
                        # exact).  Each block's carry operand is rounded to
                        # f32r ONCE from exact f32 operands — a single-ulp
                        # error per use, instead of the biased random walk a
                        # 63-step rounded chain accumulates.
                        if j == 0:
                            s_j = xk               # s_0 view; no op
                        else:
                            s_new = sp.tile([P, C], F32, tag="s")
                            nc.vector.tensor_add(s_new[:], s_prev, xk)
                            s_j = s_new[:]
                        s_prev = s_j
                        if k == NB - 1:
                            pass                   # no more carries needed
                        elif k == 0:
                            pass                   # carry (z0, xl) already set
                        elif W is None:
                            # wave 0: counts are still small enough that a
                            # single f32r rounding of the carry breaks the
                            # rel-err floor — split s_j exactly into hi+lo
                            # (one extra carry matmul for these 7 blocks)
                            sh = zp.tile([P, C], F32R, tag="z")
                            cast_r(sh[:], s_j)
                            sl = xrp.tile([P, C], F32R, tag="xr")
                            nc.vector.tensor_sub(sl[:], s_j,
                                                 sh[:].bitcast(F32))
                            carry = (sh, sl)
                            if j == G - 1:
                                W = s_j            # exact wave-0 sum (view)
                        elif j < G - 1:
                            zr = zp.tile([P, C], F32R, tag="z")
                            zr_add(zr[:], W, s_j)
                            carry = (zr,)
                        else:
                            # wave boundary: W += s_7 (exact) and round the
                            # new W for the next wave's first carry
                            zr = zp.tile([P, C], F32R, tag="z")
                            W_new = wp.tile([P, C], F32, tag="w")
                            nc.vector.tensor_add(W_new[:], W, s_j)
                            zr_add(zr[:], W, s_j)
                            W = W_new[:]
                            carry = (zr,)
                        # evacuate PSUM with the 1/(t+1) scale, narrow to bf16
                        evac(ow[:, j, :], ps[:], r_t[:, k:k + 1])
                    ov = o_d[w * G * P:(w + 1) * G * P, :].rearrange(
                        "(j p) c -> p j c", p=P)
                    # store per half-wave so the DMA starts 4 blocks earlier;
                    # quarter the final stores (cold start only) to shorten
                    # the drain — in the loop it is steady-state overhead
                    if w == NW - 1 and loop_n == 1:
                        for q in range(G // 2):
                            eng = (nc.sync if store_mode == "alt" and q % 2
                                   else nc.scalar)
                            eng.dma_start(ov[:, 2 * q:2 * q + 2, :],
                                          ow[:, 2 * q:2 * q + 2, :])
                    elif store_mode == "wave":
                        nc.scalar.dma_start(ov[:], ow[:])
                    elif store_mode == "alt":
                        nc.scalar.dma_start(ov[:, :H, :], ow[:, :H, :])
                        nc.sync.dma_start(ov[:, H:, :], ow[:, H:, :])
                    else:
                        nc.scalar.dma_start(ov[:, :H, :], ow[:, :H, :])
                        nc.scalar.dma_start(ov[:, H:, :], ow[:, H:, :])
            loop_ctx.close()

    nc.compile()
    return nc


def _build_v2(n_iter: int = 1, loop_n: int = 1, g: int = 8,
              out_dt: str = "bf16", evac_engine: str = "scalar",
              store_mode: str = "half", in_ring: str = "sync",
              s_mode: str = "round", round_engine: str = "gpsimd",
              w_engine: str = "vector", exact0: bool = True,
              s0_split: bool = True, w_split: bool = True,
              xin_bufs: int = 4, obufs: int = 3, sbufs: int = 4,
              srbufs: int = 4, wbufs: int = 3, psbufs: int = 8):
    """v2: carries come from the PE instead of a DVE-materialized operand.

    x is DMA'd straight into F32R tiles (the DRAM tensor is declared f32r, so
    no rounding op exists anywhere on the x path — the PE rounds internally;
    the BIR verifier accepts DMA as an F32R producer).  Per block k = (w, j):
        psum_k = J @ W_r[w]  +  J @ s_r[j-1]  +  U @ X_k      (all fp32r)
    where W_r[w] is the once-per-wave rounded total of completed waves and
    s_r[j-1] is a one-shot F32R rounding (round_engine, default gpsimd — the
    Pool engine is otherwise idle) of the exact f32 within-wave prefix chain
    kept on DVE.  DVE work per block drops to ONE f32 add; the old per-block
    zr_add and pre-cast disappear.  f32r hardware rounding is coarse
    (~bf16-level, truncating): s_mode='f32r' (chain writes F32R directly)
    compounds that rounding ~7x per wave and measured 5.5e-2 rel err — every
    F32R operand must be rounded ONCE from exact f32 values (v1's invariant).
    """
    G = g
    NW = NB // G
    nc = bacc.Bacc("TRN2", target_bir_lowering=False, debug=False,
                   num_devices=N_CORES)
    OUT_DT = BF16 if out_dt == "bf16" else F32

    x_d = nc.dram_tensor("x", [T, C], F32R, kind="ExternalInput")
    u_d = nc.dram_tensor("u", [P, P], F32, kind="ExternalInput")
    j_d = nc.dram_tensor("jm", [P, P], F32, kind="ExternalInput")
    r_d = nc.dram_tensor("recip", [P, NB], F32, kind="ExternalInput")
    o_d = nc.dram_tensor("out", [T, C], OUT_DT, kind="ExternalOutput")
    if exact0:
        # F32R inputs arrive 12-bit rounded, so an exact hi/lo split of
        # block 0 needs its rows again as true f32 (+256 KB, ~1% traffic)
        x0_d = nc.dram_tensor("x0", [P, C], F32, kind="ExternalInput")

    ACT_COPY = mybir.ActivationFunctionType.Copy
    evac_cycle = evac_engine.split(",")
    w_cycle = w_engine.split(",")
    cnt = {"evac": 0, "w": 0}

    def evac(dst, ps_ap, r_ap):
        eng = evac_cycle[cnt["evac"] % len(evac_cycle)]
        cnt["evac"] += 1
        if eng == "scalar":
            nc.scalar.activation(dst, ps_ap, ACT_COPY, scale=r_ap)
        else:
            getattr(nc, eng).tensor_scalar_mul(dst, ps_ap, r_ap)

    def w_op(op, dst, *srcs):
        eng = w_cycle[cnt["w"] % len(w_cycle)]
        cnt["w"] += 1
        if op == "add":
            getattr(nc, eng).tensor_add(dst, *srcs)
        elif eng == "scalar":
            nc.scalar.activation(dst, srcs[0], ACT_COPY)
        else:
            getattr(nc, eng).tensor_copy(dst, srcs[0])

    round_cycle = round_engine.split(",")

    def round_op(dst, src):
        eng = round_cycle[cnt.setdefault("r", 0) % len(round_cycle)]
        cnt["r"] += 1
        if eng == "scalar":
            nc.scalar.activation(dst, src, ACT_COPY)
        else:
            getattr(nc, eng).tensor_copy(dst, src)

    S_DT = F32R if s_mode == "f32r" else F32

    with tile.TileContext(nc) as tc:
        with (
            tc.tile_pool(name="consts", bufs=1) as consts,
            tc.tile_pool(name="xin", bufs=xin_bufs) as xin,
            tc.tile_pool(name="oput", bufs=obufs) as oput,
            tc.tile_pool(name="sp", bufs=sbufs) as sp,
            tc.tile_pool(name="srp", bufs=max(srbufs, 6)) as srp,
            tc.tile_pool(name="wp", bufs=wbufs) as wp,
            tc.tile_pool(name="wrp", bufs=wbufs) as wrp,
            tc.tile_pool(name="wlp", bufs=wbufs) as wlp,
            tc.tile_pool(name="ps", bufs=psbufs, space="PSUM") as psp,
        ):
            u_t = consts.tile([P, P], F32)
            j_t = consts.tile([P, P], F32)
            r_t = consts.tile([P, NB], F32)
            nc.gpsimd.dma_start(u_t[:], u_d[:])
            nc.gpsimd.dma_start(j_t[:], j_d[:])
            nc.gpsimd.dma_start(r_t[:], r_d[:])
            u_r = consts.tile([P, P], F32R)
            j_r = consts.tile([P, P], F32R)
            nc.vector.tensor_copy(u_r[:], u_t[:])
            nc.vector.tensor_copy(j_r[:], j_t[:])

            from contextlib import ExitStack
            loop_ctx = ExitStack()
            if loop_n > 1:
                loop_ctx.enter_context(tc.For_i(0, loop_n, 1))
            H = G // 2
            for _ in range(n_iter):
                W_f = None          # exact f32 sum of completed waves (AP)
                W_ops = []          # F32R carry operand(s) for J-matmuls
                for w in range(NW):
                    if w == 0 and exact0:
                        # block-0 exact path: true-f32 rows first on the ring
                        xf_t = sp.tile([P, C], F32, tag="s")
                        nc.sync.dma_start(xf_t[:], x0_d[:])
                        z0 = srp.tile([P, C], F32R, tag="sr")
                        nc.vector.tensor_copy(z0[:], xf_t[:])
                        xl = srp.tile([P, C], F32R, tag="sr")
                        nc.vector.tensor_sub(xl[:], xf_t[:],
                                             z0[:].bitcast(F32))
                    xw = xin.tile([P, G, C], F32R, tag="xw")
                    xv = x_d[w * G * P:(w + 1) * G * P, :].rearrange(
                        "(j p) c -> p j c", p=P)
                    in_eng = (nc.scalar if in_ring == "alt" and w % 2
                              else nc.sync)
                    if w == 0 and loop_n == 1:
                        for q in range(G // 2):
                            nc.sync.dma_start(xw[:, 2 * q:2 * q + 2, :],
                                              xv[:, 2 * q:2 * q + 2, :])
                    else:
                        in_eng.dma_start(xw[:], xv)
                    ow = oput.tile([P, G, C], OUT_DT, tag="ow")
                    s_prev = None     # exact f32 within-wave prefix s_{j-1}
                    s_ops = []        # F32R operand(s) carrying s_{j-1}
                    for j in range(G):
                        k = w * G + j
                        xk_r = xw[:, j, :]
                        xk_f = xw[:, j, :].bitcast(F32)
                        ps = psp.tile([P, C], F32, tag="ps")
                        # Block 0 divides by tiny counts (1..128) where the
                        # metric's denom floor bites, so a 12-bit rounding of
                        # x is too coarse — split the true-f32 x_0 exactly
                        # into hi+lo (the f32 sub is exact) and spend one
                        # extra U-matmul.  (z0, xl) then double as an EXACT
                        # carry for block 1.
                        if k == 0 and exact0:
                            u_ops = [z0[:], xl[:]]
                            xk_f = xf_t[:]   # exact f32 x_0 for the s chain
                        else:
                            u_ops = [xk_r]
                        # carry matmuls first so the PE needn't wait on the
                        # wave's X DMA to start the group
                        carry_ops = list(W_ops) + list(s_ops)
                        n_mm = len(carry_ops) + len(u_ops)
                        mm_i = 0

                        def mm(lhs, rhs):
                            nonlocal mm_i
                            nc.tensor.matmul(ps[:], lhs, rhs,
                                             start=(mm_i == 0),
                                             stop=(mm_i == n_mm - 1))
                            mm_i += 1

                        for op in carry_ops:
                            mm(j_r[:], op)
                        for op in u_ops:
                            mm(u_r[:], op)
                        # exact within-wave prefix chain (ONE DVE add per
                        # block); carry operand = one-shot rounding on the
                        # otherwise-idle round_engine.  s_0 is the raw x view
                        # (the PE rounds it internally, once).
                        if j == 0:
                            s_prev = xk_f
                            s_ops = u_ops
                        elif k == NB - 1:
                            pass             # nothing consumes s_63
                        elif s_mode == "f32r":
                            # fused round: the chain add writes F32R directly
                            # (ONE DVE op, no separate round).  ≤G-1 RNE
                            # roundings compound within a wave — measured
                            # ~1.3e-2 worst-case, inside the 2e-2 gate.
                            s_t = sp.tile([P, C], F32R, tag="s")
                            nc.vector.tensor_add(s_t[:], s_prev, xk_f)
                            s_prev = s_t[:].bitcast(F32)
                            s_ops = [s_t[:]]
                        else:
                            s_t = sp.tile([P, C], F32, tag="s")
                            nc.vector.tensor_add(s_t[:], s_prev, xk_f)
                            s_prev = s_t[:]
                            if j < G - 1:   # s_{G-1} only feeds W (f32)
                                sr_t = srp.tile([P, C], F32R, tag="sr")
                                round_op(sr_t[:], s_t[:])
                                if w == 0 and s0_split:
                                    # wave 0: carry ≈ whole prefix, so keep
                                    # it exact via hi+lo (extra J-matmul)
                                    sl_t = srp.tile([P, C], F32R, tag="sr")
                                    nc.vector.tensor_sub(
                                        sl_t[:], s_t[:],
                                        sr_t[:].bitcast(F32))
                                    s_ops = [sr_t[:], sl_t[:]]
                                else:
                                    s_ops = [sr_t[:]]
                        # wave boundary: fold the wave into W (exact f32 add)
                        # and round once for the next wave's carry matmuls;
                        # w_split spends an extra J-matmul to keep the W part
                        # of every later carry exact (cancellation in late
                        # csums amplifies carry rounding noise)
                        if j == G - 1 and w < NW - 1:
                            if W_f is None:
                                W_f = s_prev
                            else:
                                W_t = wp.tile([P, C], F32, tag="w")
                                w_op("add", W_t[:], W_f, s_prev)
                                W_f = W_t[:]
                            Wr_t = wrp.tile([P, C], F32R, tag="wr")
                            w_op("copy", Wr_t[:], W_f)
                            if w_split:
                                Wl_t = wlp.tile([P, C], F32R, tag="wl")
                                nc.vector.tensor_sub(Wl_t[:], W_f,
                                                     Wr_t[:].bitcast(F32))
                                W_ops = [Wr_t[:], Wl_t[:]]
                            else:
                                W_ops = [Wr_t[:]]
                        evac(ow[:, j, :], ps[:], r_t[:, k:k + 1])
                    ov = o_d[w * G * P:(w + 1) * G * P, :].rearrange(
                        "(j p) c -> p j c", p=P)
                    if w == NW - 1 and loop_n == 1:
                        for q in range(G // 2):
                            eng = (nc.sync if store_mode == "alt" and q % 2
                                   else nc.scalar)
                            eng.dma_start(ov[:, 2 * q:2 * q + 2, :],
                                          ow[:, 2 * q:2 * q + 2, :])
                    elif store_mode == "wave":
                        nc.scalar.dma_start(ov[:], ow[:])
                    elif store_mode == "alt":
                        nc.scalar.dma_start(ov[:, :H, :], ow[:, :H, :])
                        nc.sync.dma_start(ov[:, H:, :], ow[:, H:, :])
                    else:
                        nc.scalar.dma_start(ov[:, :H, :], ow[:, :H, :])
                        nc.scalar.dma_start(ov[:, H:, :], ow[:, H:, :])
            loop_ctx.close()

    nc.compile()
    return nc


def make_consts():
    s = np.arange(P)
    u = (s[:, None] <= s[None, :]).astype(np.float32)          # u[s,t]=1 if s<=t
    jm = np.ones((P, P), dtype=np.float32)
    counts = (np.arange(NB)[None, :] * P + s[:, None] + 1)     # [P, NB]
    recip = (1.0 / counts).astype(np.float32)
    return u, jm, recip


def program_input_names(nc):
    import concourse.mybir as _mb
    names = set()
    for alloc in nc.m.functions[0].allocations:
        if (isinstance(alloc, _mb.MemoryLocationSet)
                and alloc.kind == "ExternalInput"):
            names.add(alloc.memorylocations[0].name)
    return names


def make_in_maps(nc, x):
    """Per-core input dicts for run_bass_kernel_spmd, matching nc's inputs."""
    u, jm, recip = make_consts()
    names = program_input_names(nc)
    maps = []
    for b in range(N_CORES):
        m = {"x": x[b], "u": u, "jm": jm, "recip": recip}
        if "x0" in names:
            m["x0"] = np.ascontiguousarray(x[b][:P])
        maps.append(m)
    return maps


def kernel(x):
    x = np.ascontiguousarray(np.asarray(x), dtype=np.float32)
    assert x.shape == (B, T, C), x.shape
    if "nc" not in _cache:
        _cache["nc"] = build_program()
    nc = _cache["nc"]
    res = run_bass_kernel_spmd(nc, make_in_maps(nc, x),
                               list(range(N_CORES)))
    out = np.stack([np.asarray(res.results[b]["out"], dtype=np.float32)
                    for b in range(N_CORES)], axis=0)
    return out

